# revision 1
# baseline (speedup 1.0000x reference)
"""Data-parallel Trainium kernel for nn_AblatedGATTransformer.

Strategy (per sharding hint): pure data parallel — shard batch B=512 across
8 NeuronCores (64 graphs-sequences per core); all parameters replicated.
The GAT edge-softmax (reference uses segment_max/segment_sum over an edge
list) is rewritten as dense masked 5x5 attention: the edge list only enters
through a [N,N] multiplicity/count matrix and mask, which are precomputed
host-side from edge_index (exactly equivalent, including duplicate edges
and the implicit self-loops the reference appends).
"""

import math

import numpy as np
import jax
import jax.numpy as jnp

B, S, N, F = 512, 72, 5, 10
H, C = 4, 32
D = H * C
L, FF = 3, 256
K, NS = 6, 5
EPS = 1e-5
M_CORES = 8
B_LOC = B // M_CORES


def _layer_norm(x, g, b):
    m = x.mean(-1, keepdims=True)
    v = ((x - m) ** 2).mean(-1, keepdims=True)
    return (x - m) / jnp.sqrt(v + EPS) * g + b


def _forward(x, cnt, negmask, pe, gat_w, gat_att_src, gat_att_dst, gat_bias,
             gat_ln_g, gat_ln_b, qkv_w, qkv_b, out_w, out_b,
             ln1_g, ln1_b, ff1_w, ff1_b, ff2_w, ff2_b, ln2_g, ln2_b,
             h1_w, h1_b, h2_w, h2_b):
    """x: [B_LOC, S, N, F]. Dense-graph GAT + transformer + heads."""
    G = B_LOC * S
    xf = x.reshape(G, N, F)

    # GAT node projection
    h = jnp.einsum('gnf,fhc->gnhc', xf, gat_w.reshape(F, H, C))  # [G,N,H,C]
    a_src = jnp.einsum('gnhc,hc->gnh', h, gat_att_src)           # [G,N,H]
    a_dst = jnp.einsum('gnhc,hc->gnh', h, gat_att_dst)

    # Dense edge scores e[g,s,d,h] over all (src,dst) pairs
    e = a_src[:, :, None, :] + a_dst[:, None, :, :]              # [G,S=N,D=N,H]
    e = jax.nn.leaky_relu(e, 0.2)
    # masked per-destination softmax over sources, with edge multiplicities
    m = jnp.max(e + negmask[None, :, :, None], axis=1, keepdims=True)
    ex = jnp.exp(e - m) * cnt[None, :, :, None]                  # [G,N,N,H]
    den = ex.sum(axis=1)                                         # [G,N(dst),H]
    agg = jnp.einsum('gsdh,gshc->gdhc', ex, h) / den[..., None]  # [G,N,H,C]

    gat_out = agg.reshape(G, N, D) + gat_bias
    gat_out = _layer_norm(gat_out, gat_ln_g, gat_ln_b)

    # mean-pool nodes -> temporal sequence, add positional encoding
    t = gat_out.reshape(B_LOC, S, N, D).mean(axis=2) + pe        # [B,S,D]

    scale = 1.0 / math.sqrt(D // H)
    for l in range(L):
        qkv = t @ qkv_w[l].T + qkv_b[l]
        q, k, v = jnp.split(qkv, 3, axis=-1)
        q = q.reshape(B_LOC, S, H, D // H)
        k = k.reshape(B_LOC, S, H, D // H)
        v = v.reshape(B_LOC, S, H, D // H)
        scores = jnp.einsum('bqhd,bkhd->bhqk', q, k) * scale
        att = jax.nn.softmax(scores, axis=-1)
        o = jnp.einsum('bhqk,bkhd->bqhd', att, v).reshape(B_LOC, S, D)
        o = o @ out_w[l].T + out_b[l]
        t = _layer_norm(t + o, ln1_g[l], ln1_b[l])
        f = jax.nn.relu(t @ ff1_w[l].T + ff1_b[l]) @ ff2_w[l].T + ff2_b[l]
        t = _layer_norm(t + f, ln2_g[l], ln2_b[l])

    x_last = t[:, -1, :]
    h1 = jax.nn.relu(jnp.einsum('bd,kod->kbo', x_last, h1_w) + h1_b[:, None, :])
    out = jnp.einsum('kbo,kpo->kbp', h1, h2_w) + h2_b[:, None, :]
    return out  # [K, B_LOC, NS]


_PMAPPED = None


def _get_pmapped():
    global _PMAPPED
    if _PMAPPED is None:
        _PMAPPED = jax.pmap(_forward, axis_name='cores',
                            devices=jax.devices()[:M_CORES])
    return _PMAPPED


def _edge_tables(edge_index: np.ndarray):
    """Dense multiplicity count [N,N] (src,dst) incl. self-loops + -inf mask."""
    cnt = np.zeros((N, N), np.float32)
    src, dst = np.asarray(edge_index[0]), np.asarray(edge_index[1])
    for s, d in zip(src, dst):
        cnt[int(s), int(d)] += 1.0
    for i in range(N):
        cnt[i, i] += 1.0  # reference appends self loops
    negmask = np.where(cnt > 0, 0.0, -1e30).astype(np.float32)
    return cnt, negmask


def _pe_table():
    pos = np.arange(S, dtype=np.float32)[:, None]
    div = np.exp(np.arange(0, D, 2, dtype=np.float32) * (-math.log(10000.0) / D))
    pe = np.zeros((S, D), np.float32)
    pe[:, 0::2] = np.sin(pos * div)
    pe[:, 1::2] = np.cos(pos * div)
    return pe


def kernel(**inputs) -> np.ndarray:
    x = np.asarray(inputs['x'], np.float32)           # [B,S,N,F]
    edge_index = np.asarray(inputs['edge_index'])

    cnt, negmask = _edge_tables(edge_index)
    pe = _pe_table()

    # Shard x across cores along batch; replicate everything else.
    xs = x.reshape(M_CORES, B_LOC, S, N, F)

    def rep(a):
        a = np.asarray(a, np.float32)
        return np.broadcast_to(a, (M_CORES,) + a.shape)

    param_names = ['gat_w', 'gat_att_src', 'gat_att_dst', 'gat_bias',
                   'gat_ln_g', 'gat_ln_b', 'qkv_w', 'qkv_b', 'out_w', 'out_b',
                   'ln1_g', 'ln1_b', 'ff1_w', 'ff1_b', 'ff2_w', 'ff2_b',
                   'ln2_g', 'ln2_b', 'h1_w', 'h1_b', 'h2_w', 'h2_b']
    params = [rep(inputs[p]) for p in param_names]

    fn = _get_pmapped()
    out = fn(xs, rep(cnt), rep(negmask), rep(pe), *params)
    out = np.asarray(jax.device_get(out))             # [M, K, B_LOC, NS]
    out = np.concatenate([out[i] for i in range(M_CORES)], axis=1)
    return out.astype(np.float32)                     # [K, B, NS]


if __name__ == '__main__':
    rng = np.random.default_rng(0)
    demo = {
        'x': rng.standard_normal((B, S, N, F), dtype=np.float32),
        'edge_index': np.stack(np.nonzero(1 - np.eye(N, dtype=np.int32))).astype(np.int32),
        'gat_w': rng.standard_normal((F, D), dtype=np.float32) * 0.05,
        'gat_att_src': rng.standard_normal((H, C), dtype=np.float32) * 0.05,
        'gat_att_dst': rng.standard_normal((H, C), dtype=np.float32) * 0.05,
        'gat_bias': np.zeros(D, np.float32),
        'gat_ln_g': np.ones(D, np.float32),
        'gat_ln_b': np.zeros(D, np.float32),
        'qkv_w': rng.standard_normal((L, 3 * D, D), dtype=np.float32) * 0.05,
        'qkv_b': np.zeros((L, 3 * D), np.float32),
        'out_w': rng.standard_normal((L, D, D), dtype=np.float32) * 0.05,
        'out_b': np.zeros((L, D), np.float32),
        'ln1_g': np.ones((L, D), np.float32),
        'ln1_b': np.zeros((L, D), np.float32),
        'ff1_w': rng.standard_normal((L, FF, D), dtype=np.float32) * 0.05,
        'ff1_b': np.zeros((L, FF), np.float32),
        'ff2_w': rng.standard_normal((L, D, FF), dtype=np.float32) * 0.05,
        'ff2_b': np.zeros((L, D), np.float32),
        'ln2_g': np.ones((L, D), np.float32),
        'ln2_b': np.zeros((L, D), np.float32),
        'h1_w': rng.standard_normal((K, D // 2, D), dtype=np.float32) * 0.05,
        'h1_b': np.zeros((K, D // 2), np.float32),
        'h2_w': rng.standard_normal((K, NS, D // 2), dtype=np.float32) * 0.05,
        'h2_b': np.zeros((K, NS), np.float32),
    }
    print(kernel(**demo).shape)

